# revision 36
# baseline (speedup 1.0000x reference)
"""LIF spiking-neuron scan on 8 Trainium2 NeuronCores.

Reference semantics (bit-exact replication):
    mem_t = v_decay * mem_{t-1} * (1 - spk_{t-1}) + x_t
    spk_t = ((mem_t / (v_th + 1e-8)) - 1 > 0)          # for v_th > 0

Device: two fused scalar_tensor_tensor DVE instructions per timestep
    u_t   = (mem_t is_le C2) mult mem_t        # reset mask (exact {0,1})
    mem_t = (u_{t-1} mult d) add x_t           # decay + input
(each ALU op rounds to f32 exactly like the reference's separate jax
ops; the {0,1} mask multiply is exact). C2 is the f32 bit-boundary of
the reference's threshold predicate, found host-side by bisection.
(A single fused custom-DVE op of the whole step lowers fine but this
container's walrus rejects CUSTOM_DVE_ANT encodings — "ISA wrong
length" — so the 2-op form is used.)

The Scalar engine (ACT) converts each produced mem block to a spike
code s8 = Sign(mem - C2) in fp8_e4m3 ({-1, 0, +1} are exact in e4m3)
and streams s8 out on its own HWDGE queue -> output DMA is 4x smaller.
Host decode: spk = (s8 > 0)  [exact: Sign(mem-C2)=+1  <=>  mem > C2].

Sharding: batch dim (64) split 8 ways -> per core [T=100, 8, 4096] =
[128 partitions, 100*256 f32] in a time-major transposed DRAM layout.
The whole per-core input (100KB/partition) persists in SBUF; loads are
issued upfront on the Sync engine's HWDGE queue with one completion
semaphore per chunk.
"""

import os

import numpy as np

import concourse.bass as bass
import concourse.mybir as mybir
from concourse.bass_utils import run_bass_kernel_spmd

T, B, N = 100, 64, 4096
NCORES = 8
P = 128
BPC = B // NCORES          # batch rows per core
FD = BPC * N // P          # 256 free elems per partition per timestep
F32 = mybir.dt.float32
F8 = mybir.dt.float8e4

EPS = np.float32(1e-8)

BLOCKS = [2, 3, 4, 5, 6] + [10] * 7 + [5, 3, 1, 1]
assert sum(BLOCKS) == T
KMAX = max(BLOCKS)
RING = 6   # ob ring slots (each KMAX steps)
NF32TAIL = 2  # last blocks stored as raw f32 mem (fp8 mode only)

# Debug/validation knobs (harness never sets these):
#  LIF_CHAIN=1  -> per-op engine self-semaphore chains (CoreSim race detector)
#  LIF_F32OUT=0 -> fp8 Sign-code output via ACT (measured slower: ACT tail)
_USE_CHAIN = os.environ.get("LIF_CHAIN", "0") == "1"
_F32OUT = os.environ.get("LIF_F32OUT", "1") == "1"
_TRACE = bool(os.environ.get("LIF_TRACE"))
LAST_RUN = None  # BassKernelResults of the most recent run (for test.py)


# ---------------------------------------------------------------------------
# Custom fused DVE op registration (runtime-append to concourse.dve_ops.OPS)
# ---------------------------------------------------------------------------
_REGISTERED = {}


def _register_lif_op(ge_mask: bool):
    """LIF_STEP_(LE|GE): out = (Src0 cmp C0) * Src0 * C1 + Src1."""
    name = "LIF_STEP_GE" if ge_mask else "LIF_STEP_LE"
    if name in _REGISTERED:
        return _REGISTERED[name]
    import concourse.dve_ops as dops
    from concourse.dve_spec import Spec, Src0, Src1, C0, C1, lower
    from concourse.dve_uop import DveOpSpec

    if ge_mask:
        body = (Src0 >= C0) * Src0 * C1 + Src1
        ref = lambda in0, in1, s0, s1, imm2: (
            (in0 * (in0 >= s0).astype(np.float32)).astype(np.float32)
            * np.float32(s1) + in1).astype(np.float32)
    else:
        body = (Src0 <= C0) * Src0 * C1 + Src1
        ref = lambda in0, in1, s0, s1, imm2: (
            (in0 * (in0 <= s0).astype(np.float32)).astype(np.float32)
            * np.float32(s1) + in1).astype(np.float32)
    spec = Spec(body=body, reference=ref)

    # pin the sha by computing it (same function the compile-time check runs)
    row = dops._CUSTOM_DVE_ROW_BASE + len(dops.OPS)
    assert row < 0x20, "custom-DVE opcode rows exhausted"
    shas = {}
    for ver in ("v3", "v4"):
        shas[ver] = DveOpSpec(
            name=name, opcode=row, uops=lower(spec, ver=ver), rd1_en=True
        ).sha(ver)
    op = dops.DveOp(name, spec, subdim=False, uops_sha=shas)
    dops.OPS.append(op)
    dops.CUSTOM_DVE_SPECS[name] = spec
    dops._SUB_OPCODE_FOR_NAME[name] = row
    _REGISTERED[name] = op
    return op


# ---------------------------------------------------------------------------
# Threshold boundary (host-side, exact)
# ---------------------------------------------------------------------------
def _predicate(vth: np.float32):
    """Return (pred(m)->bool, increasing: bool) replicating the reference's
    mem_thr > 0 in f32."""
    c = np.float32(vth + EPS)
    assert c != 0.0, "degenerate threshold"
    one = np.float32(1.0)
    if vth > 0:
        pred = lambda m: (np.float32(np.float32(m) / c) - one) > 0
        increasing = True
    else:
        pred = lambda m: (one - np.float32(np.float32(m) / c)) > 0
        # m/c decreasing in m for c<0 -> 1-m/c increasing; c>0 -> decreasing
        increasing = c < 0
    return pred, increasing


def _f32_key(m) -> int:
    """Map f32 to an int key monotone in the float total order."""
    i = int(np.frombuffer(np.float32(m).tobytes(), np.uint32)[0])
    return i ^ 0xFFFFFFFF if i & 0x80000000 else i | 0x80000000


def _key_f32(k: int):
    u = (k & 0x7FFFFFFF) if k & 0x80000000 else (k ^ 0xFFFFFFFF)
    return np.frombuffer(np.uint32(u).tobytes(), np.float32)[0]


def spike_boundary(vth: np.float32):
    """Find the exact f32 boundary of the spike predicate.

    Returns (b, spk_is_gt):
      if spk_is_gt:  spk = (mem > b),  device no-spike mask = (mem is_le b)
      else:          spk = (mem < b),  device no-spike mask = (mem is_ge b)
    """
    with np.errstate(over="ignore"):
        pred, increasing = _predicate(vth)
        lo_k, hi_k = _f32_key(np.float32(-3.4e38)), _f32_key(np.float32(3.4e38))
        if increasing:
            assert not pred(_key_f32(lo_k)) and pred(_key_f32(hi_k))
            while hi_k - lo_k > 1:  # find max m with pred false
                mid = (lo_k + hi_k) // 2
                if pred(_key_f32(mid)):
                    hi_k = mid
                else:
                    lo_k = mid
            b = _key_f32(lo_k)
            assert not pred(b) and pred(_key_f32(lo_k + 1))
            return b, True
        else:
            assert pred(_key_f32(lo_k)) and not pred(_key_f32(hi_k))
            while hi_k - lo_k > 1:  # find min m with pred false
                mid = (lo_k + hi_k) // 2
                if pred(_key_f32(mid)):
                    lo_k = mid
                else:
                    hi_k = mid
            b = _key_f32(hi_k)
            assert not pred(b) and pred(_key_f32(hi_k - 1))
            return b, False


# ---------------------------------------------------------------------------
# Device program
# ---------------------------------------------------------------------------
def build_program(c2: float, d: float, spk_is_gt: bool) -> bass.Bass:
    nc = bass.Bass("TRN2", target_bir_lowering=False, debug=False,
                   enable_asserts=False)
    x_d = nc.dram_tensor("x", [P, T * FD], F32, kind="ExternalInput")
    NBL = len(BLOCKS)
    starts = [sum(BLOCKS[:i]) for i in range(NBL)]
    tail_steps = sum(BLOCKS[-NF32TAIL:])
    tail_start = T - tail_steps
    if _F32OUT:
        m_d = nc.dram_tensor("m", [P, T * FD], F32, kind="ExternalOutput")
    else:
        m_d = nc.dram_tensor("m8", [P, T * FD], F8, kind="ExternalOutput")
        mt_d = nc.dram_tensor("mtail", [P, tail_steps * FD], F32,
                              kind="ExternalOutput")
        nb_d = nc.dram_tensor("nbias", [P, 1], F32, kind="ExternalInput")
    mask_op = mybir.AluOpType.is_le if spk_is_gt else mybir.AluOpType.is_ge

    xb = nc.alloc_sbuf_tensor("xb", [P, T * FD], F32)            # whole input
    ob = nc.alloc_sbuf_tensor("ob", [P, RING * KMAX * FD], F32)  # mem ring
    uu = nc.alloc_sbuf_tensor("uu", [P, FD], F32)
    if not _F32OUT:
        s8 = nc.alloc_sbuf_tensor("s8", [P, T * FD], F8)         # spike codes
        # Sign bias (-c2): loaded from a tiny input, registered as const AP
        cst = nc.alloc_sbuf_tensor("const-lif-bias", [P, 1], F32)
        nc.const_aps.aps[(F32, -float(c2))] = cst.ap()

    xc = [nc.alloc_semaphore(f"xc{b}") for b in range(NBL)]  # per-chunk load
    dvc = nc.alloc_semaphore("dvc")   # DVE self-chain (LIF_CHAIN mode only)
    acc = nc.alloc_semaphore("acc")   # ACT self-chain (LIF_CHAIN mode only)
    mrd = nc.alloc_semaphore("mrd")   # DVE produced block (1/blk)
    asg = nc.alloc_semaphore("asg")   # ACT signed block (1/blk)
    bsm = nc.alloc_semaphore("bsm")   # bias const loaded
    msl = [nc.alloc_semaphore(f"msl{r}") for r in range(RING)]  # per-slot store

    with nc.Block() as blk:

        @blk.sync
        def _(sync):
            # stream the whole input into SBUF; chunks never reused
            for b in range(NBL):
                lo, L = starts[b] * FD, BLOCKS[b] * FD
                sync.dma_start(
                    xb[:, lo:lo + L], x_d[:, lo:lo + L]
                ).then_inc(xc[b], 16)
                if b == 0 and not _F32OUT:
                    sync.dma_start(cst[:, :], nb_d[:, :]).then_inc(bsm, 16)

        @blk.scalar
        def _(act):
            na = 0
            if not _F32OUT:
                act.wait_ge(bsm, 16)  # bias const loaded

            def achain(ins):
                nonlocal na
                if _USE_CHAIN:
                    if na > 0:
                        ins._wait_ge(acc, na)
                    ins.then_inc(acc, 1)
                na += 1
                return ins

            nsg = 0  # signed-block counter
            for b in range(NBL):
                lo, L = starts[b] * FD, BLOCKS[b] * FD
                slot = (b % RING) * KMAX * FD
                act.wait_ge(mrd, b + 1)
                if _F32OUT:
                    act.dma_start(
                        m_d[:, lo:lo + L], ob[:, slot:slot + L]
                    ).then_inc(msl[b % RING], 16)
                elif b >= NBL - NF32TAIL:
                    # tail blocks: store raw f32 mem (no Sign lag at the end)
                    tlo = (starts[b] - tail_start) * FD
                    act.dma_start(
                        mt_d[:, tlo:tlo + L], ob[:, slot:slot + L]
                    ).then_inc(msl[b % RING], 16)
                else:
                    last = None
                    for kk in range(BLOCKS[b]):
                        t = starts[b] + kk
                        last = achain(act.activation(
                            s8[:, t * FD:(t + 1) * FD],
                            ob[:, slot + kk * FD: slot + (kk + 1) * FD],
                            mybir.ActivationFunctionType.Sign,
                            bias=-float(c2), scale=1.0,
                        ))
                    nsg += 1
                    if _USE_CHAIN:
                        act.sem_inc(asg, 1)._wait_ge(acc, na)
                    else:
                        last.then_inc(asg, 1)
                    act.dma_start(
                        m_d[:, lo:lo + L], s8[:, lo:lo + L]
                    ).then_inc(msl[b % RING], 16)._wait_ge(asg, nsg)
            for r in range(RING):
                n_r = len([bb for bb in range(NBL) if bb % RING == r])
                act.wait_ge(msl[r], 16 * n_r)

        @blk.vector
        def _(v):
            prev_mem = None
            nv = 0

            def chain(ins):
                nonlocal nv
                if _USE_CHAIN:
                    if nv > 0:
                        ins._wait_ge(dvc, nv)
                    ins.then_inc(dvc, 1)
                nv += 1
                return ins

            for b in range(NBL):
                slot = (b % RING) * KMAX * FD
                # block gates (standalone; compute ops carry the chain wait)
                if b > 0:
                    v.wait_ge(xc[b], 16)                 # chunk b loaded
                if b >= RING:
                    # ring slot consumer finished with block b-RING
                    if _F32OUT:
                        v.wait_ge(msl[b % RING], 16 * (b // RING))
                    else:
                        v.wait_ge(asg, b - RING + 1)
                last = None
                for kk in range(BLOCKS[b]):
                    t = starts[b] + kk
                    mslice = ob[:, slot + kk * FD: slot + (kk + 1) * FD]
                    xslice = xb[:, t * FD:(t + 1) * FD]
                    if t == 0:
                        # mem_0 = x_0 (u_{-1} = 0); gate on chunk-0 load
                        last = chain(v.tensor_copy(mslice, xslice))
                        last._wait_ge(xc[0], 16)
                    else:
                        chain(v.scalar_tensor_tensor(
                            uu[:], prev_mem, float(c2), prev_mem,
                            mask_op, mybir.AluOpType.mult,
                        ))
                        last = chain(v.scalar_tensor_tensor(
                            mslice, uu[:], float(d), xslice,
                            mybir.AluOpType.mult, mybir.AluOpType.add,
                        ))
                    prev_mem = mslice
                # block-produced signal fires when the last op COMPLETES
                # (DVE datapath completes in issue order)
                if _USE_CHAIN:
                    v.sem_inc(mrd, 1)._wait_ge(dvc, nv)
                else:
                    last.then_inc(mrd, 1)

    return nc


_PROGRAM_CACHE: dict = {}


def kernel(inpt: np.ndarray, v_th: np.ndarray, v_decay: np.ndarray) -> np.ndarray:
    global LAST_RUN
    x = np.ascontiguousarray(np.asarray(inpt, dtype=np.float32))
    assert x.shape == (T, B, N), x.shape
    vth = np.float32(np.asarray(v_th))
    d = float(np.float32(np.asarray(v_decay)))
    b, spk_is_gt = spike_boundary(vth)

    key = (float(b), d, spk_is_gt, _F32OUT, _USE_CHAIN)
    if key not in _PROGRAM_CACHE:
        _PROGRAM_CACHE[key] = build_program(float(b), d, spk_is_gt)
    nc = _PROGRAM_CACHE[key]

    in_maps = []
    nbias = np.full((P, 1), -np.float32(b), dtype=np.float32)
    for k in range(NCORES):
        xk = x[:, k * BPC:(k + 1) * BPC, :].reshape(T, P, FD)
        xk = np.ascontiguousarray(xk.transpose(1, 0, 2)).reshape(P, T * FD)
        m = {"x": xk}
        if not _F32OUT:
            m["nbias"] = nbias
        in_maps.append(m)

    res = run_bass_kernel_spmd(
        nc, in_maps, core_ids=list(range(NCORES)), trace=_TRACE
    )
    LAST_RUN = res

    tail_steps = sum(BLOCKS[-NF32TAIL:])
    t0 = T - tail_steps
    spikes = np.empty((T, B, N), dtype=np.float32)
    for k in range(NCORES):
        if _F32OUT:
            mem = res.results[k]["m"].reshape(P, T, FD).transpose(1, 0, 2)
            mem = mem.reshape(T, BPC, N)
            cmp = (mem > b) if spk_is_gt else (mem < b)
        else:
            s = res.results[k]["m8"].astype(np.float32)
            s = s.reshape(P, T, FD).transpose(1, 0, 2).reshape(T, BPC, N)
            cmp = (s > 0) if spk_is_gt else (s < 0)
            mt = res.results[k]["mtail"].reshape(P, tail_steps, FD)
            mt = mt.transpose(1, 0, 2).reshape(tail_steps, BPC, N)
            cmp[t0:] = (mt > b) if spk_is_gt else (mt < b)
        spikes[:, k * BPC:(k + 1) * BPC, :] = cmp
    return spikes


# revision 38
# speedup vs baseline: 1.1571x; 1.1571x over previous
"""LIF spiking-neuron scan on 8 Trainium2 NeuronCores.

Reference semantics (bit-exact replication):
    mem_t = v_decay * mem_{t-1} * (1 - spk_{t-1}) + x_t
    spk_t = ((mem_t / (v_th + 1e-8)) - 1 > 0)          # for v_th > 0

Device: two fused scalar_tensor_tensor DVE instructions per timestep
    u_t   = (mem_t is_le C2) mult mem_t        # reset mask (exact {0,1})
    mem_t = (u_{t-1} mult d) add x_t           # decay + input
(each ALU op rounds to f32 exactly like the reference's separate jax
ops; the {0,1} mask multiply is exact). C2 is the f32 bit-boundary of
the reference's threshold predicate, found host-side by bisection.
(A single fused custom-DVE op of the whole step lowers fine but this
container's walrus rejects CUSTOM_DVE_ANT encodings — "ISA wrong
length" — so the 2-op form is used.)

The Scalar engine (ACT) converts each produced mem block to a spike
code s8 = Sign(mem - C2) in fp8_e4m3 ({-1, 0, +1} are exact in e4m3)
and streams s8 out on its own HWDGE queue -> output DMA is 4x smaller.
Host decode: spk = (s8 > 0)  [exact: Sign(mem-C2)=+1  <=>  mem > C2].

Sharding: batch dim (64) split 8 ways -> per core [T=100, 8, 4096] =
[128 partitions, 100*256 f32] in a time-major transposed DRAM layout.
The whole per-core input (100KB/partition) persists in SBUF; loads are
issued upfront on the Sync engine's HWDGE queue with one completion
semaphore per chunk.
"""

import os

import numpy as np

import concourse.bass as bass
import concourse.mybir as mybir
from concourse.bass_utils import run_bass_kernel_spmd

T, B, N = 100, 64, 4096
NCORES = 8
P = 128
BPC = B // NCORES          # batch rows per core
FD = BPC * N // P          # 256 free elems per partition per timestep
F32 = mybir.dt.float32
F8 = mybir.dt.float8e4

EPS = np.float32(1e-8)

BLOCKS = [2, 3, 4, 5, 6] + [10] * 7 + [6, 3, 1]
assert sum(BLOCKS) == T
KMAX = max(BLOCKS)
RING = 6   # ob ring slots (each KMAX steps)
NF32TAIL = 2  # last blocks stored as raw f32 mem (fp8 mode only)

# Debug/validation knobs (harness never sets these):
#  LIF_CHAIN=1  -> per-op engine self-semaphore chains (CoreSim race detector)
#  LIF_F32OUT=0 -> fp8 Sign-code output via ACT (measured slower: ACT tail)
_USE_CHAIN = os.environ.get("LIF_CHAIN", "0") == "1"
_F32OUT = os.environ.get("LIF_F32OUT", "1") == "1"
_TRACE = bool(os.environ.get("LIF_TRACE"))
LAST_RUN = None  # BassKernelResults of the most recent run (for test.py)


# ---------------------------------------------------------------------------
# Custom fused DVE op registration (runtime-append to concourse.dve_ops.OPS)
# ---------------------------------------------------------------------------
_REGISTERED = {}


def _register_lif_op(ge_mask: bool):
    """LIF_STEP_(LE|GE): out = (Src0 cmp C0) * Src0 * C1 + Src1."""
    name = "LIF_STEP_GE" if ge_mask else "LIF_STEP_LE"
    if name in _REGISTERED:
        return _REGISTERED[name]
    import concourse.dve_ops as dops
    from concourse.dve_spec import Spec, Src0, Src1, C0, C1, lower
    from concourse.dve_uop import DveOpSpec

    if ge_mask:
        body = (Src0 >= C0) * Src0 * C1 + Src1
        ref = lambda in0, in1, s0, s1, imm2: (
            (in0 * (in0 >= s0).astype(np.float32)).astype(np.float32)
            * np.float32(s1) + in1).astype(np.float32)
    else:
        body = (Src0 <= C0) * Src0 * C1 + Src1
        ref = lambda in0, in1, s0, s1, imm2: (
            (in0 * (in0 <= s0).astype(np.float32)).astype(np.float32)
            * np.float32(s1) + in1).astype(np.float32)
    spec = Spec(body=body, reference=ref)

    # pin the sha by computing it (same function the compile-time check runs)
    row = dops._CUSTOM_DVE_ROW_BASE + len(dops.OPS)
    assert row < 0x20, "custom-DVE opcode rows exhausted"
    shas = {}
    for ver in ("v3", "v4"):
        shas[ver] = DveOpSpec(
            name=name, opcode=row, uops=lower(spec, ver=ver), rd1_en=True
        ).sha(ver)
    op = dops.DveOp(name, spec, subdim=False, uops_sha=shas)
    dops.OPS.append(op)
    dops.CUSTOM_DVE_SPECS[name] = spec
    dops._SUB_OPCODE_FOR_NAME[name] = row
    _REGISTERED[name] = op
    return op


# ---------------------------------------------------------------------------
# Threshold boundary (host-side, exact)
# ---------------------------------------------------------------------------
def _predicate(vth: np.float32):
    """Return (pred(m)->bool, increasing: bool) replicating the reference's
    mem_thr > 0 in f32."""
    c = np.float32(vth + EPS)
    assert c != 0.0, "degenerate threshold"
    one = np.float32(1.0)
    if vth > 0:
        pred = lambda m: (np.float32(np.float32(m) / c) - one) > 0
        increasing = True
    else:
        pred = lambda m: (one - np.float32(np.float32(m) / c)) > 0
        # m/c decreasing in m for c<0 -> 1-m/c increasing; c>0 -> decreasing
        increasing = c < 0
    return pred, increasing


def _f32_key(m) -> int:
    """Map f32 to an int key monotone in the float total order."""
    i = int(np.frombuffer(np.float32(m).tobytes(), np.uint32)[0])
    return i ^ 0xFFFFFFFF if i & 0x80000000 else i | 0x80000000


def _key_f32(k: int):
    u = (k & 0x7FFFFFFF) if k & 0x80000000 else (k ^ 0xFFFFFFFF)
    return np.frombuffer(np.uint32(u).tobytes(), np.float32)[0]


def spike_boundary(vth: np.float32):
    """Find the exact f32 boundary of the spike predicate.

    Returns (b, spk_is_gt):
      if spk_is_gt:  spk = (mem > b),  device no-spike mask = (mem is_le b)
      else:          spk = (mem < b),  device no-spike mask = (mem is_ge b)
    """
    with np.errstate(over="ignore"):
        pred, increasing = _predicate(vth)
        lo_k, hi_k = _f32_key(np.float32(-3.4e38)), _f32_key(np.float32(3.4e38))
        if increasing:
            assert not pred(_key_f32(lo_k)) and pred(_key_f32(hi_k))
            while hi_k - lo_k > 1:  # find max m with pred false
                mid = (lo_k + hi_k) // 2
                if pred(_key_f32(mid)):
                    hi_k = mid
                else:
                    lo_k = mid
            b = _key_f32(lo_k)
            assert not pred(b) and pred(_key_f32(lo_k + 1))
            return b, True
        else:
            assert pred(_key_f32(lo_k)) and not pred(_key_f32(hi_k))
            while hi_k - lo_k > 1:  # find min m with pred false
                mid = (lo_k + hi_k) // 2
                if pred(_key_f32(mid)):
                    lo_k = mid
                else:
                    hi_k = mid
            b = _key_f32(hi_k)
            assert not pred(b) and pred(_key_f32(hi_k - 1))
            return b, False


# ---------------------------------------------------------------------------
# Device program
# ---------------------------------------------------------------------------
def build_program(c2: float, d: float, spk_is_gt: bool) -> bass.Bass:
    nc = bass.Bass("TRN2", target_bir_lowering=False, debug=False,
                   enable_asserts=False)
    x_d = nc.dram_tensor("x", [P, T * FD], F32, kind="ExternalInput")
    NBL = len(BLOCKS)
    starts = [sum(BLOCKS[:i]) for i in range(NBL)]
    tail_steps = sum(BLOCKS[-NF32TAIL:])
    tail_start = T - tail_steps
    if _F32OUT:
        m_d = nc.dram_tensor("m", [P, T * FD], F32, kind="ExternalOutput")
    else:
        m_d = nc.dram_tensor("m8", [P, T * FD], F8, kind="ExternalOutput")
        mt_d = nc.dram_tensor("mtail", [P, tail_steps * FD], F32,
                              kind="ExternalOutput")
        nb_d = nc.dram_tensor("nbias", [P, 1], F32, kind="ExternalInput")
    mask_op = mybir.AluOpType.is_le if spk_is_gt else mybir.AluOpType.is_ge

    xb = nc.alloc_sbuf_tensor("xb", [P, T * FD], F32)            # whole input
    ob = nc.alloc_sbuf_tensor("ob", [P, RING * KMAX * FD], F32)  # mem ring
    uu = nc.alloc_sbuf_tensor("uu", [P, FD], F32)
    if not _F32OUT:
        s8 = nc.alloc_sbuf_tensor("s8", [P, T * FD], F8)         # spike codes
        # Sign bias (-c2): loaded from a tiny input, registered as const AP
        cst = nc.alloc_sbuf_tensor("const-lif-bias", [P, 1], F32)
        nc.const_aps.aps[(F32, -float(c2))] = cst.ap()

    xc = [nc.alloc_semaphore(f"xc{b}") for b in range(NBL)]  # per-chunk load
    dvc = nc.alloc_semaphore("dvc")   # DVE self-chain (LIF_CHAIN mode only)
    acc = nc.alloc_semaphore("acc")   # ACT self-chain (LIF_CHAIN mode only)
    mrd = nc.alloc_semaphore("mrd")   # DVE produced block (1/blk)
    asg = nc.alloc_semaphore("asg")   # ACT signed block (1/blk)
    bsm = nc.alloc_semaphore("bsm")   # bias const loaded
    msl = [nc.alloc_semaphore(f"msl{r}") for r in range(RING)]  # per-slot store

    with nc.Block() as blk:

        @blk.sync
        def _(sync):
            # stream the whole input into SBUF; chunks never reused
            for b in range(NBL):
                lo, L = starts[b] * FD, BLOCKS[b] * FD
                sync.dma_start(
                    xb[:, lo:lo + L], x_d[:, lo:lo + L]
                ).then_inc(xc[b], 16)
                if b == 0 and not _F32OUT:
                    sync.dma_start(cst[:, :], nb_d[:, :]).then_inc(bsm, 16)

        @blk.scalar
        def _(act):
            na = 0
            if not _F32OUT:
                act.wait_ge(bsm, 16)  # bias const loaded

            def achain(ins):
                nonlocal na
                if _USE_CHAIN:
                    if na > 0:
                        ins._wait_ge(acc, na)
                    ins.then_inc(acc, 1)
                na += 1
                return ins

            nsg = 0  # signed-block counter
            for b in range(NBL):
                lo, L = starts[b] * FD, BLOCKS[b] * FD
                slot = (b % RING) * KMAX * FD
                act.wait_ge(mrd, b + 1)
                if _F32OUT:
                    act.dma_start(
                        m_d[:, lo:lo + L], ob[:, slot:slot + L]
                    ).then_inc(msl[b % RING], 16)
                elif b >= NBL - NF32TAIL:
                    # tail blocks: store raw f32 mem (no Sign lag at the end)
                    tlo = (starts[b] - tail_start) * FD
                    act.dma_start(
                        mt_d[:, tlo:tlo + L], ob[:, slot:slot + L]
                    ).then_inc(msl[b % RING], 16)
                else:
                    last = None
                    for kk in range(BLOCKS[b]):
                        t = starts[b] + kk
                        last = achain(act.activation(
                            s8[:, t * FD:(t + 1) * FD],
                            ob[:, slot + kk * FD: slot + (kk + 1) * FD],
                            mybir.ActivationFunctionType.Sign,
                            bias=-float(c2), scale=1.0,
                        ))
                    nsg += 1
                    if _USE_CHAIN:
                        act.sem_inc(asg, 1)._wait_ge(acc, na)
                    else:
                        last.then_inc(asg, 1)
                    act.dma_start(
                        m_d[:, lo:lo + L], s8[:, lo:lo + L]
                    ).then_inc(msl[b % RING], 16)._wait_ge(asg, nsg)
            for r in range(RING):
                n_r = len([bb for bb in range(NBL) if bb % RING == r])
                act.wait_ge(msl[r], 16 * n_r)

        @blk.vector
        def _(v):
            prev_mem = None
            nv = 0

            def chain(ins):
                nonlocal nv
                if _USE_CHAIN:
                    if nv > 0:
                        ins._wait_ge(dvc, nv)
                    ins.then_inc(dvc, 1)
                nv += 1
                return ins

            for b in range(NBL):
                slot = (b % RING) * KMAX * FD
                # block gates: chunk-b loaded; ring-slot consumer done with
                # block b-RING. In no-chain mode these ride the free wait
                # slot of the block's first two ops (keeps the DVE stream at
                # exactly 199 instructions); chain mode uses standalone waits.
                pend = []
                if b > 0:
                    pend.append((xc[b], 16))
                if b >= RING:
                    if _F32OUT:
                        pend.append((msl[b % RING], 16 * (b // RING)))
                    else:
                        pend.append((asg, b - RING + 1))
                if _USE_CHAIN:
                    for sem_, val_ in pend:
                        v.wait_ge(sem_, val_)
                    pend = []
                last = None
                for kk in range(BLOCKS[b]):
                    t = starts[b] + kk
                    mslice = ob[:, slot + kk * FD: slot + (kk + 1) * FD]
                    xslice = xb[:, t * FD:(t + 1) * FD]
                    if t == 0:
                        # mem_0 = x_0 (u_{-1} = 0); gate on chunk-0 load
                        last = chain(v.tensor_copy(mslice, xslice))
                        last._wait_ge(xc[0], 16)
                    else:
                        ins1 = chain(v.scalar_tensor_tensor(
                            uu[:], prev_mem, float(c2), prev_mem,
                            mask_op, mybir.AluOpType.mult,
                        ))
                        if pend:
                            sem_, val_ = pend.pop(0)
                            ins1._wait_ge(sem_, val_)
                        last = chain(v.scalar_tensor_tensor(
                            mslice, uu[:], float(d), xslice,
                            mybir.AluOpType.mult, mybir.AluOpType.add,
                        ))
                        if pend:
                            sem_, val_ = pend.pop(0)
                            last._wait_ge(sem_, val_)
                    prev_mem = mslice
                # block-produced signal fires when the last op COMPLETES
                # (DVE datapath completes in issue order)
                if _USE_CHAIN:
                    v.sem_inc(mrd, 1)._wait_ge(dvc, nv)
                else:
                    last.then_inc(mrd, 1)

    return nc


_PROGRAM_CACHE: dict = {}


def kernel(inpt: np.ndarray, v_th: np.ndarray, v_decay: np.ndarray) -> np.ndarray:
    global LAST_RUN
    x = np.ascontiguousarray(np.asarray(inpt, dtype=np.float32))
    assert x.shape == (T, B, N), x.shape
    vth = np.float32(np.asarray(v_th))
    d = float(np.float32(np.asarray(v_decay)))
    b, spk_is_gt = spike_boundary(vth)

    key = (float(b), d, spk_is_gt, _F32OUT, _USE_CHAIN)
    if key not in _PROGRAM_CACHE:
        _PROGRAM_CACHE[key] = build_program(float(b), d, spk_is_gt)
    nc = _PROGRAM_CACHE[key]

    in_maps = []
    nbias = np.full((P, 1), -np.float32(b), dtype=np.float32)
    for k in range(NCORES):
        xk = x[:, k * BPC:(k + 1) * BPC, :].reshape(T, P, FD)
        xk = np.ascontiguousarray(xk.transpose(1, 0, 2)).reshape(P, T * FD)
        m = {"x": xk}
        if not _F32OUT:
            m["nbias"] = nbias
        in_maps.append(m)

    res = run_bass_kernel_spmd(
        nc, in_maps, core_ids=list(range(NCORES)), trace=_TRACE
    )
    LAST_RUN = res

    tail_steps = sum(BLOCKS[-NF32TAIL:])
    t0 = T - tail_steps
    spikes = np.empty((T, B, N), dtype=np.float32)
    for k in range(NCORES):
        if _F32OUT:
            mem = res.results[k]["m"].reshape(P, T, FD).transpose(1, 0, 2)
            mem = mem.reshape(T, BPC, N)
            cmp = (mem > b) if spk_is_gt else (mem < b)
        else:
            s = res.results[k]["m8"].astype(np.float32)
            s = s.reshape(P, T, FD).transpose(1, 0, 2).reshape(T, BPC, N)
            cmp = (s > 0) if spk_is_gt else (s < 0)
            mt = res.results[k]["mtail"].reshape(P, tail_steps, FD)
            mt = mt.transpose(1, 0, 2).reshape(tail_steps, BPC, N)
            cmp[t0:] = (mt > b) if spk_is_gt else (mt < b)
        spikes[:, k * BPC:(k + 1) * BPC, :] = cmp
    return spikes


# revision 43
# speedup vs baseline: 1.1808x; 1.0205x over previous
"""LIF spiking-neuron scan on 8 Trainium2 NeuronCores.

Reference semantics (bit-exact replication):
    mem_t = v_decay * mem_{t-1} * (1 - spk_{t-1}) + x_t
    spk_t = ((mem_t / (v_th + 1e-8)) - 1 > 0)          # for v_th > 0

Device: two fused scalar_tensor_tensor DVE instructions per timestep
    u_t   = (mem_t is_le C2) mult mem_t        # reset mask (exact {0,1})
    mem_t = (u_{t-1} mult d) add x_t           # decay + input
(each ALU op rounds to f32 exactly like the reference's separate jax
ops; the {0,1} mask multiply is exact). C2 is the f32 bit-boundary of
the reference's threshold predicate, found host-side by bisection.
(A single fused custom-DVE op of the whole step lowers fine but this
container's walrus rejects CUSTOM_DVE_ANT encodings — "ISA wrong
length" — so the 2-op form is used.)

The Scalar engine (ACT) converts each produced mem block to a spike
code s8 = Sign(mem - C2) in fp8_e4m3 ({-1, 0, +1} are exact in e4m3)
and streams s8 out on its own HWDGE queue -> output DMA is 4x smaller.
Host decode: spk = (s8 > 0)  [exact: Sign(mem-C2)=+1  <=>  mem > C2].

Sharding: batch dim (64) split 8 ways -> per core [T=100, 8, 4096] =
[128 partitions, 100*256 f32] in a time-major transposed DRAM layout.
The whole per-core input (100KB/partition) persists in SBUF; loads are
issued upfront on the Sync engine's HWDGE queue with one completion
semaphore per chunk.
"""

import os

import numpy as np

import concourse.bass as bass
import concourse.mybir as mybir
from concourse.bass_utils import run_bass_kernel_spmd

T, B, N = 100, 64, 4096
NCORES = 8
P = 128
BPC = B // NCORES          # batch rows per core
FD = BPC * N // P          # 256 free elems per partition per timestep
F32 = mybir.dt.float32
F8 = mybir.dt.float8e4

EPS = np.float32(1e-8)

BLOCKS = [2, 3, 4, 5, 6] + [10] * 7 + [6, 3, 1]
assert sum(BLOCKS) == T
KMAX = max(BLOCKS)
RING = 6   # ob ring slots (each KMAX steps)
NF32TAIL = 2  # last blocks stored as raw f32 mem (fp8 mode only)

# Debug/validation knobs (harness never sets these):
#  LIF_CHAIN=1  -> per-op engine self-semaphore chains (CoreSim race detector)
#  LIF_F32OUT=1 -> output raw mem f32 (no ACT Sign path)
#  LIF_NOFUSE=1 -> two scalar_tensor_tensor ops/step instead of the fused op
_USE_CHAIN = os.environ.get("LIF_CHAIN", "0") == "1"
_F32OUT = os.environ.get("LIF_F32OUT", "0") == "1"
_FUSE = os.environ.get("LIF_NOFUSE", "0") != "1"
_TRACE = bool(os.environ.get("LIF_TRACE"))
LAST_RUN = None  # BassKernelResults of the most recent run (for test.py)


# ---------------------------------------------------------------------------
# Custom fused DVE op registration (runtime-append to concourse.dve_ops.OPS)
# ---------------------------------------------------------------------------
_REGISTERED = {}


def _register_lif_op(ge_mask: bool):
    """LIF_STEP_(LE|GE): out = (Src0 cmp C0) * Src0 * C1 + Src1."""
    name = "LIF_STEP_GE" if ge_mask else "LIF_STEP_LE"
    if name in _REGISTERED:
        return _REGISTERED[name]
    import concourse.dve_ops as dops
    from concourse.dve_spec import Spec, Src0, Src1, C0, C1, lower
    from concourse.dve_uop import DveOpSpec

    if ge_mask:
        body = (Src0 >= C0) * Src0 * C1 + Src1
        ref = lambda in0, in1, s0, s1, imm2: (
            (in0 * (in0 >= s0).astype(np.float32)).astype(np.float32)
            * np.float32(s1) + in1).astype(np.float32)
    else:
        body = (Src0 <= C0) * Src0 * C1 + Src1
        ref = lambda in0, in1, s0, s1, imm2: (
            (in0 * (in0 <= s0).astype(np.float32)).astype(np.float32)
            * np.float32(s1) + in1).astype(np.float32)
    spec = Spec(body=body, reference=ref)

    # pin the sha by computing it (same function the compile-time check runs)
    row = dops._CUSTOM_DVE_ROW_BASE + len(dops.OPS)
    assert row < 0x20, "custom-DVE opcode rows exhausted"
    shas = {}
    for ver in ("v3", "v4"):
        shas[ver] = DveOpSpec(
            name=name, opcode=row, uops=lower(spec, ver=ver), rd1_en=True
        ).sha(ver)
    op = dops.DveOp(name, spec, subdim=False, uops_sha=shas)
    dops.OPS.append(op)
    dops.CUSTOM_DVE_SPECS[name] = spec
    dops._SUB_OPCODE_FOR_NAME[name] = row
    _REGISTERED[name] = op
    return op


# ---------------------------------------------------------------------------
# Threshold boundary (host-side, exact)
# ---------------------------------------------------------------------------
def _predicate(vth: np.float32):
    """Return (pred(m)->bool, increasing: bool) replicating the reference's
    mem_thr > 0 in f32."""
    c = np.float32(vth + EPS)
    assert c != 0.0, "degenerate threshold"
    one = np.float32(1.0)
    if vth > 0:
        pred = lambda m: (np.float32(np.float32(m) / c) - one) > 0
        increasing = True
    else:
        pred = lambda m: (one - np.float32(np.float32(m) / c)) > 0
        # m/c decreasing in m for c<0 -> 1-m/c increasing; c>0 -> decreasing
        increasing = c < 0
    return pred, increasing


def _f32_key(m) -> int:
    """Map f32 to an int key monotone in the float total order."""
    i = int(np.frombuffer(np.float32(m).tobytes(), np.uint32)[0])
    return i ^ 0xFFFFFFFF if i & 0x80000000 else i | 0x80000000


def _key_f32(k: int):
    u = (k & 0x7FFFFFFF) if k & 0x80000000 else (k ^ 0xFFFFFFFF)
    return np.frombuffer(np.uint32(u).tobytes(), np.float32)[0]


def spike_boundary(vth: np.float32):
    """Find the exact f32 boundary of the spike predicate.

    Returns (b, spk_is_gt):
      if spk_is_gt:  spk = (mem > b),  device no-spike mask = (mem is_le b)
      else:          spk = (mem < b),  device no-spike mask = (mem is_ge b)
    """
    with np.errstate(over="ignore"):
        pred, increasing = _predicate(vth)
        lo_k, hi_k = _f32_key(np.float32(-3.4e38)), _f32_key(np.float32(3.4e38))
        if increasing:
            assert not pred(_key_f32(lo_k)) and pred(_key_f32(hi_k))
            while hi_k - lo_k > 1:  # find max m with pred false
                mid = (lo_k + hi_k) // 2
                if pred(_key_f32(mid)):
                    hi_k = mid
                else:
                    lo_k = mid
            b = _key_f32(lo_k)
            assert not pred(b) and pred(_key_f32(lo_k + 1))
            return b, True
        else:
            assert pred(_key_f32(lo_k)) and not pred(_key_f32(hi_k))
            while hi_k - lo_k > 1:  # find min m with pred false
                mid = (lo_k + hi_k) // 2
                if pred(_key_f32(mid)):
                    lo_k = mid
                else:
                    hi_k = mid
            b = _key_f32(hi_k)
            assert not pred(b) and pred(_key_f32(hi_k - 1))
            return b, False


# ---------------------------------------------------------------------------
# Device program
# ---------------------------------------------------------------------------
def build_program(c2: float, d: float, spk_is_gt: bool) -> bass.Bass:
    lif_op = _register_lif_op(ge_mask=not spk_is_gt) if _FUSE else None
    nc = bass.Bass("TRN2", target_bir_lowering=False, debug=False,
                   enable_asserts=False)
    x_d = nc.dram_tensor("x", [P, T * FD], F32, kind="ExternalInput")
    NBL = len(BLOCKS)
    starts = [sum(BLOCKS[:i]) for i in range(NBL)]
    tail_steps = sum(BLOCKS[-NF32TAIL:])
    tail_start = T - tail_steps
    if _F32OUT:
        m_d = nc.dram_tensor("m", [P, T * FD], F32, kind="ExternalOutput")
    else:
        m_d = nc.dram_tensor("m8", [P, T * FD], F8, kind="ExternalOutput")
        mt_d = nc.dram_tensor("mtail", [P, tail_steps * FD], F32,
                              kind="ExternalOutput")
        nb_d = nc.dram_tensor("nbias", [P, 1], F32, kind="ExternalInput")
    mask_op = mybir.AluOpType.is_le if spk_is_gt else mybir.AluOpType.is_ge

    xb = nc.alloc_sbuf_tensor("xb", [P, T * FD], F32)            # whole input
    ob = nc.alloc_sbuf_tensor("ob", [P, RING * KMAX * FD], F32)  # mem ring
    uu = nc.alloc_sbuf_tensor("uu", [P, FD], F32)
    if not _F32OUT:
        s8 = nc.alloc_sbuf_tensor("s8", [P, T * FD], F8)         # spike codes
        # Sign bias (-c2): loaded from a tiny input, registered as const AP
        cst = nc.alloc_sbuf_tensor("const-lif-bias", [P, 1], F32)
        nc.const_aps.aps[(F32, -float(c2))] = cst.ap()

    xc = [nc.alloc_semaphore(f"xc{b}") for b in range(NBL)]  # per-chunk load
    dvc = nc.alloc_semaphore("dvc")   # DVE self-chain (LIF_CHAIN mode only)
    acc = nc.alloc_semaphore("acc")   # ACT self-chain (LIF_CHAIN mode only)
    mrd = nc.alloc_semaphore("mrd")   # DVE produced block (1/blk)
    asg = nc.alloc_semaphore("asg")   # ACT signed block (1/blk)
    bsm = nc.alloc_semaphore("bsm")   # bias const loaded
    msl = [nc.alloc_semaphore(f"msl{r}") for r in range(RING)]  # per-slot store

    with nc.Block() as blk:

        @blk.sync
        def _(sync):
            # stream the whole input into SBUF; chunks never reused
            for b in range(NBL):
                lo, L = starts[b] * FD, BLOCKS[b] * FD
                sync.dma_start(
                    xb[:, lo:lo + L], x_d[:, lo:lo + L]
                ).then_inc(xc[b], 16)
                if b == 0 and not _F32OUT:
                    sync.dma_start(cst[:, :], nb_d[:, :]).then_inc(bsm, 16)

        @blk.scalar
        def _(act):
            na = 0
            if not _F32OUT:
                act.wait_ge(bsm, 16)  # bias const loaded

            def achain(ins):
                nonlocal na
                if _USE_CHAIN:
                    if na > 0:
                        ins._wait_ge(acc, na)
                    ins.then_inc(acc, 1)
                na += 1
                return ins

            nsg = 0  # signed-block counter
            for b in range(NBL):
                lo, L = starts[b] * FD, BLOCKS[b] * FD
                slot = (b % RING) * KMAX * FD
                act.wait_ge(mrd, b + 1)
                if _F32OUT:
                    act.dma_start(
                        m_d[:, lo:lo + L], ob[:, slot:slot + L]
                    ).then_inc(msl[b % RING], 16)
                elif b >= NBL - NF32TAIL:
                    # tail blocks: store raw f32 mem (no Sign lag at the end)
                    tlo = (starts[b] - tail_start) * FD
                    act.dma_start(
                        mt_d[:, tlo:tlo + L], ob[:, slot:slot + L]
                    ).then_inc(msl[b % RING], 16)
                else:
                    last = None
                    for kk in range(BLOCKS[b]):
                        t = starts[b] + kk
                        last = achain(act.activation(
                            s8[:, t * FD:(t + 1) * FD],
                            ob[:, slot + kk * FD: slot + (kk + 1) * FD],
                            mybir.ActivationFunctionType.Sign,
                            bias=-float(c2), scale=1.0,
                        ))
                    nsg += 1
                    if _USE_CHAIN:
                        act.sem_inc(asg, 1)._wait_ge(acc, na)
                    else:
                        last.then_inc(asg, 1)
                    act.dma_start(
                        m_d[:, lo:lo + L], s8[:, lo:lo + L]
                    ).then_inc(msl[b % RING], 16)._wait_ge(asg, nsg)
            for r in range(RING):
                n_r = len([bb for bb in range(NBL) if bb % RING == r])
                act.wait_ge(msl[r], 16 * n_r)

        @blk.vector
        def _(v):
            prev_mem = None
            nv = 0

            def chain(ins):
                nonlocal nv
                if _USE_CHAIN:
                    if nv > 0:
                        ins._wait_ge(dvc, nv)
                    ins.then_inc(dvc, 1)
                nv += 1
                return ins

            for b in range(NBL):
                slot = (b % RING) * KMAX * FD
                # block gates: chunk-b loaded; ring-slot consumer done with
                # block b-RING. In no-chain mode these ride free wait slots
                # on existing ops (1 sync-wait max per compute op): the slot
                # gate goes on the block's first op; the chunk-b gate rides
                # the PREVIOUS block's last op in fused mode (where a block's
                # first step is a single instruction), or the first step's
                # second op in 2-op mode. Chain mode uses standalone waits.
                pend = []
                if b > 0 and (_USE_CHAIN or not _FUSE):
                    pend.append((xc[b], 16))
                if b >= RING:
                    if _F32OUT:
                        pend.append((msl[b % RING], 16 * (b // RING)))
                    else:
                        pend.append((asg, b - RING + 1))
                if _USE_CHAIN:
                    for sem_, val_ in pend:
                        v.wait_ge(sem_, val_)
                    pend = []
                last = None
                for kk in range(BLOCKS[b]):
                    t = starts[b] + kk
                    mslice = ob[:, slot + kk * FD: slot + (kk + 1) * FD]
                    xslice = xb[:, t * FD:(t + 1) * FD]
                    if t == 0:
                        # mem_0 = x_0 (u_{-1} = 0); gate on chunk-0 load
                        last = chain(v.tensor_copy(mslice, xslice))
                        last._wait_ge(xc[0], 16)
                    elif _FUSE:
                        last = chain(v._custom_dve(
                            lif_op, out=mslice, in0=prev_mem, in1=xslice,
                            s0=float(c2), s1=float(d),
                        ))
                        if pend:
                            sem_, val_ = pend.pop(0)
                            last._wait_ge(sem_, val_)
                    else:
                        ins1 = chain(v.scalar_tensor_tensor(
                            uu[:], prev_mem, float(c2), prev_mem,
                            mask_op, mybir.AluOpType.mult,
                        ))
                        if pend:
                            sem_, val_ = pend.pop(0)
                            ins1._wait_ge(sem_, val_)
                        last = chain(v.scalar_tensor_tensor(
                            mslice, uu[:], float(d), xslice,
                            mybir.AluOpType.mult, mybir.AluOpType.add,
                        ))
                        if pend:
                            sem_, val_ = pend.pop(0)
                            last._wait_ge(sem_, val_)
                    prev_mem = mslice
                # block-produced signal fires when the last op COMPLETES
                # (DVE datapath completes in issue order)
                if _USE_CHAIN:
                    v.sem_inc(mrd, 1)._wait_ge(dvc, nv)
                else:
                    last.then_inc(mrd, 1)
                    if _FUSE and b + 1 < NBL:
                        # chunk b+1 gate rides this block's last op
                        last._wait_ge(xc[b + 1], 16)

    # pack .instr bytes for InstISA subclasses (custom-DVE); raw Bass
    # doesn't run this pass and walrus rejects empty blobs
    mybir.codegen_inst_isa_subclasses(nc)
    return nc


_PROGRAM_CACHE: dict = {}


def kernel(inpt: np.ndarray, v_th: np.ndarray, v_decay: np.ndarray) -> np.ndarray:
    global LAST_RUN
    x = np.ascontiguousarray(np.asarray(inpt, dtype=np.float32))
    assert x.shape == (T, B, N), x.shape
    vth = np.float32(np.asarray(v_th))
    d = float(np.float32(np.asarray(v_decay)))
    b, spk_is_gt = spike_boundary(vth)

    key = (float(b), d, spk_is_gt, _F32OUT, _USE_CHAIN, _FUSE)
    if key not in _PROGRAM_CACHE:
        _PROGRAM_CACHE[key] = build_program(float(b), d, spk_is_gt)
    nc = _PROGRAM_CACHE[key]

    in_maps = []
    nbias = np.full((P, 1), -np.float32(b), dtype=np.float32)
    for k in range(NCORES):
        xk = x[:, k * BPC:(k + 1) * BPC, :].reshape(T, P, FD)
        xk = np.ascontiguousarray(xk.transpose(1, 0, 2)).reshape(P, T * FD)
        m = {"x": xk}
        if not _F32OUT:
            m["nbias"] = nbias
        in_maps.append(m)

    res = run_bass_kernel_spmd(
        nc, in_maps, core_ids=list(range(NCORES)), trace=_TRACE
    )
    LAST_RUN = res

    tail_steps = sum(BLOCKS[-NF32TAIL:])
    t0 = T - tail_steps
    spikes = np.empty((T, B, N), dtype=np.float32)
    for k in range(NCORES):
        if _F32OUT:
            mem = res.results[k]["m"].reshape(P, T, FD).transpose(1, 0, 2)
            mem = mem.reshape(T, BPC, N)
            cmp = (mem > b) if spk_is_gt else (mem < b)
        else:
            s = res.results[k]["m8"].astype(np.float32)
            s = s.reshape(P, T, FD).transpose(1, 0, 2).reshape(T, BPC, N)
            cmp = (s > 0) if spk_is_gt else (s < 0)
            mt = res.results[k]["mtail"].reshape(P, tail_steps, FD)
            mt = mt.transpose(1, 0, 2).reshape(tail_steps, BPC, N)
            cmp[t0:] = (mt > b) if spk_is_gt else (mt < b)
        spikes[:, k * BPC:(k + 1) * BPC, :] = cmp
    return spikes


# revision 44
# speedup vs baseline: 1.3572x; 1.1493x over previous
"""LIF spiking-neuron scan on 8 Trainium2 NeuronCores.

Reference semantics (bit-exact replication):
    mem_t = v_decay * mem_{t-1} * (1 - spk_{t-1}) + x_t
    spk_t = ((mem_t / (v_th + 1e-8)) - 1 > 0)          # for v_th > 0

Device: two fused scalar_tensor_tensor DVE instructions per timestep
    u_t   = (mem_t is_le C2) mult mem_t        # reset mask (exact {0,1})
    mem_t = (u_{t-1} mult d) add x_t           # decay + input
(each ALU op rounds to f32 exactly like the reference's separate jax
ops; the {0,1} mask multiply is exact). C2 is the f32 bit-boundary of
the reference's threshold predicate, found host-side by bisection.
(A single fused custom-DVE op of the whole step lowers fine but this
container's walrus rejects CUSTOM_DVE_ANT encodings — "ISA wrong
length" — so the 2-op form is used.)

The Scalar engine (ACT) converts each produced mem block to a spike
code s8 = Sign(mem - C2) in fp8_e4m3 ({-1, 0, +1} are exact in e4m3)
and streams s8 out on its own HWDGE queue -> output DMA is 4x smaller.
Host decode: spk = (s8 > 0)  [exact: Sign(mem-C2)=+1  <=>  mem > C2].

Sharding: batch dim (64) split 8 ways -> per core [T=100, 8, 4096] =
[128 partitions, 100*256 f32] in a time-major transposed DRAM layout.
The whole per-core input (100KB/partition) persists in SBUF; loads are
issued upfront on the Sync engine's HWDGE queue with one completion
semaphore per chunk.
"""

import os

import numpy as np

import concourse.bass as bass
import concourse.mybir as mybir
from concourse.bass_utils import run_bass_kernel_spmd

T, B, N = 100, 64, 4096
NCORES = 8
P = 128
BPC = B // NCORES          # batch rows per core
FD = BPC * N // P          # 256 free elems per partition per timestep
F32 = mybir.dt.float32
F8 = mybir.dt.float8e4

EPS = np.float32(1e-8)

BLOCKS = [2, 3, 4, 5, 6] + [10] * 7 + [6, 3, 1]
assert sum(BLOCKS) == T
KMAX = max(BLOCKS)
RING = 6   # ob ring slots (each KMAX steps)
NF32TAIL = 2  # last blocks stored as raw f32 mem (fp8 mode only)

# Debug/validation knobs (harness never sets these):
#  LIF_CHAIN=1  -> per-op engine self-semaphore chains (CoreSim race detector)
#  LIF_F32OUT=1 -> output raw mem f32 (no ACT Sign path)
#  LIF_NOFUSE=1 -> two scalar_tensor_tensor ops/step instead of the fused op
_USE_CHAIN = os.environ.get("LIF_CHAIN", "0") == "1"
_F32OUT = os.environ.get("LIF_F32OUT", "0") == "1"
_FUSE = os.environ.get("LIF_NOFUSE", "0") != "1"
_TRACE = bool(os.environ.get("LIF_TRACE"))
LAST_RUN = None  # BassKernelResults of the most recent run (for test.py)


# ---------------------------------------------------------------------------
# Custom fused DVE op registration (runtime-append to concourse.dve_ops.OPS)
# ---------------------------------------------------------------------------
_REGISTERED = {}


def _register_lif_op(ge_mask: bool):
    """LIF_STEP_(LE|GE): out = (Src0 cmp C0) * Src0 * C1 + Src1."""
    name = "LIF_STEP_GE" if ge_mask else "LIF_STEP_LE"
    if name in _REGISTERED:
        return _REGISTERED[name]
    import concourse.dve_ops as dops
    from concourse.dve_spec import Spec, Src0, Src1, C0, C1, lower
    from concourse.dve_uop import DveOpSpec

    if ge_mask:
        body = (Src0 >= C0) * Src0 * C1 + Src1
        ref = lambda in0, in1, s0, s1, imm2: (
            (in0 * (in0 >= s0).astype(np.float32)).astype(np.float32)
            * np.float32(s1) + in1).astype(np.float32)
    else:
        body = (Src0 <= C0) * Src0 * C1 + Src1
        ref = lambda in0, in1, s0, s1, imm2: (
            (in0 * (in0 <= s0).astype(np.float32)).astype(np.float32)
            * np.float32(s1) + in1).astype(np.float32)
    spec = Spec(body=body, reference=ref)

    # pin the sha by computing it (same function the compile-time check runs)
    row = dops._CUSTOM_DVE_ROW_BASE + len(dops.OPS)
    assert row < 0x20, "custom-DVE opcode rows exhausted"
    shas = {}
    for ver in ("v3", "v4"):
        shas[ver] = DveOpSpec(
            name=name, opcode=row, uops=lower(spec, ver=ver), rd1_en=True
        ).sha(ver)
    op = dops.DveOp(name, spec, subdim=False, uops_sha=shas)
    dops.OPS.append(op)
    dops.CUSTOM_DVE_SPECS[name] = spec
    dops._SUB_OPCODE_FOR_NAME[name] = row
    _REGISTERED[name] = op
    return op


# ---------------------------------------------------------------------------
# Threshold boundary (host-side, exact)
# ---------------------------------------------------------------------------
def _predicate(vth: np.float32):
    """Return (pred(m)->bool, increasing: bool) replicating the reference's
    mem_thr > 0 in f32."""
    c = np.float32(vth + EPS)
    assert c != 0.0, "degenerate threshold"
    one = np.float32(1.0)
    if vth > 0:
        pred = lambda m: (np.float32(np.float32(m) / c) - one) > 0
        increasing = True
    else:
        pred = lambda m: (one - np.float32(np.float32(m) / c)) > 0
        # m/c decreasing in m for c<0 -> 1-m/c increasing; c>0 -> decreasing
        increasing = c < 0
    return pred, increasing


def _f32_key(m) -> int:
    """Map f32 to an int key monotone in the float total order."""
    i = int(np.frombuffer(np.float32(m).tobytes(), np.uint32)[0])
    return i ^ 0xFFFFFFFF if i & 0x80000000 else i | 0x80000000


def _key_f32(k: int):
    u = (k & 0x7FFFFFFF) if k & 0x80000000 else (k ^ 0xFFFFFFFF)
    return np.frombuffer(np.uint32(u).tobytes(), np.float32)[0]


def spike_boundary(vth: np.float32):
    """Find the exact f32 boundary of the spike predicate.

    Returns (b, spk_is_gt):
      if spk_is_gt:  spk = (mem > b),  device no-spike mask = (mem is_le b)
      else:          spk = (mem < b),  device no-spike mask = (mem is_ge b)
    """
    with np.errstate(over="ignore"):
        pred, increasing = _predicate(vth)
        lo_k, hi_k = _f32_key(np.float32(-3.4e38)), _f32_key(np.float32(3.4e38))
        if increasing:
            assert not pred(_key_f32(lo_k)) and pred(_key_f32(hi_k))
            while hi_k - lo_k > 1:  # find max m with pred false
                mid = (lo_k + hi_k) // 2
                if pred(_key_f32(mid)):
                    hi_k = mid
                else:
                    lo_k = mid
            b = _key_f32(lo_k)
            assert not pred(b) and pred(_key_f32(lo_k + 1))
            return b, True
        else:
            assert pred(_key_f32(lo_k)) and not pred(_key_f32(hi_k))
            while hi_k - lo_k > 1:  # find min m with pred false
                mid = (lo_k + hi_k) // 2
                if pred(_key_f32(mid)):
                    lo_k = mid
                else:
                    hi_k = mid
            b = _key_f32(hi_k)
            assert not pred(b) and pred(_key_f32(hi_k - 1))
            return b, False


# ---------------------------------------------------------------------------
# Device program
# ---------------------------------------------------------------------------
def build_program(c2: float, d: float, spk_is_gt: bool) -> bass.Bass:
    lif_op = _register_lif_op(ge_mask=not spk_is_gt) if _FUSE else None
    nc = bass.Bass("TRN2", target_bir_lowering=False, debug=False,
                   enable_asserts=False)
    x_d = nc.dram_tensor("x", [P, T * FD], F32, kind="ExternalInput")
    NBL = len(BLOCKS)
    starts = [sum(BLOCKS[:i]) for i in range(NBL)]
    tail_steps = sum(BLOCKS[-NF32TAIL:])
    tail_start = T - tail_steps
    if _F32OUT:
        m_d = nc.dram_tensor("m", [P, T * FD], F32, kind="ExternalOutput")
    else:
        m_d = nc.dram_tensor("m8", [P, T * FD], F8, kind="ExternalOutput")
        mt_d = nc.dram_tensor("mtail", [P, tail_steps * FD], F32,
                              kind="ExternalOutput")
        nb_d = nc.dram_tensor("nbias", [P, 1], F32, kind="ExternalInput")
    mask_op = mybir.AluOpType.is_le if spk_is_gt else mybir.AluOpType.is_ge

    xb = nc.alloc_sbuf_tensor("xb", [P, T * FD], F32)            # whole input
    ob = nc.alloc_sbuf_tensor("ob", [P, RING * KMAX * FD], F32)  # mem ring
    uu = nc.alloc_sbuf_tensor("uu", [P, FD], F32)
    if not _F32OUT:
        s8 = nc.alloc_sbuf_tensor("s8", [P, T * FD], F8)         # spike codes
        # Sign bias (-c2): loaded from a tiny input, registered as const AP
        cst = nc.alloc_sbuf_tensor("const-lif-bias", [P, 1], F32)
        nc.const_aps.aps[(F32, -float(c2))] = cst.ap()

    xc = [nc.alloc_semaphore(f"xc{b}") for b in range(NBL)]  # per-chunk load
    dvc = nc.alloc_semaphore("dvc")   # DVE self-chain (LIF_CHAIN mode only)
    acc = nc.alloc_semaphore("acc")   # ACT self-chain (LIF_CHAIN mode only)
    mrd = nc.alloc_semaphore("mrd")   # DVE produced block (1/blk)
    asg = nc.alloc_semaphore("asg")   # ACT signed block (1/blk)
    bsm = nc.alloc_semaphore("bsm")   # bias const loaded
    msl = [nc.alloc_semaphore(f"msl{r}") for r in range(RING)]  # per-slot store

    with nc.Block() as blk:

        @blk.sync
        def _(sync):
            # stream the whole input into SBUF; chunks never reused
            for b in range(NBL):
                lo, L = starts[b] * FD, BLOCKS[b] * FD
                sync.dma_start(
                    xb[:, lo:lo + L], x_d[:, lo:lo + L]
                ).then_inc(xc[b], 16)
                if b == 0 and not _F32OUT:
                    sync.dma_start(cst[:, :], nb_d[:, :]).then_inc(bsm, 16)

        @blk.scalar
        def _(act):
            na = 0
            if not _F32OUT:
                act.wait_ge(bsm, 16)  # bias const loaded

            def achain(ins):
                nonlocal na
                if _USE_CHAIN:
                    if na > 0:
                        ins._wait_ge(acc, na)
                    ins.then_inc(acc, 1)
                na += 1
                return ins

            nsg = 0  # signed-block counter
            for b in range(NBL):
                lo, L = starts[b] * FD, BLOCKS[b] * FD
                slot = (b % RING) * KMAX * FD
                act.wait_ge(mrd, b + 1)
                if _F32OUT:
                    act.dma_start(
                        m_d[:, lo:lo + L], ob[:, slot:slot + L]
                    ).then_inc(msl[b % RING], 16)
                elif b >= NBL - NF32TAIL:
                    # tail blocks: store raw f32 mem (no Sign lag at the end)
                    tlo = (starts[b] - tail_start) * FD
                    act.dma_start(
                        mt_d[:, tlo:tlo + L], ob[:, slot:slot + L]
                    ).then_inc(msl[b % RING], 16)
                else:
                    # one Sign over the whole contiguous block slot
                    last = achain(act.activation(
                        s8[:, lo:lo + L],
                        ob[:, slot:slot + L],
                        mybir.ActivationFunctionType.Sign,
                        bias=-float(c2), scale=1.0,
                    ))
                    nsg += 1
                    if _USE_CHAIN:
                        act.sem_inc(asg, 1)._wait_ge(acc, na)
                    else:
                        last.then_inc(asg, 1)
                    act.dma_start(
                        m_d[:, lo:lo + L], s8[:, lo:lo + L]
                    ).then_inc(msl[b % RING], 16)._wait_ge(asg, nsg)
            for r in range(RING):
                n_r = len([bb for bb in range(NBL) if bb % RING == r])
                act.wait_ge(msl[r], 16 * n_r)

        @blk.vector
        def _(v):
            prev_mem = None
            nv = 0

            def chain(ins):
                nonlocal nv
                if _USE_CHAIN:
                    if nv > 0:
                        ins._wait_ge(dvc, nv)
                    ins.then_inc(dvc, 1)
                nv += 1
                return ins

            for b in range(NBL):
                slot = (b % RING) * KMAX * FD
                # block gates: chunk-b loaded; ring-slot consumer done with
                # block b-RING. In no-chain mode these ride free wait slots
                # on existing ops (1 sync-wait max per compute op): the slot
                # gate goes on the block's first op; the chunk-b gate rides
                # the PREVIOUS block's last op in fused mode (where a block's
                # first step is a single instruction), or the first step's
                # second op in 2-op mode. Chain mode uses standalone waits.
                pend = []
                if b > 0 and (_USE_CHAIN or not _FUSE):
                    pend.append((xc[b], 16))
                if b >= RING:
                    if _F32OUT:
                        pend.append((msl[b % RING], 16 * (b // RING)))
                    else:
                        pend.append((asg, b - RING + 1))
                if _USE_CHAIN:
                    for sem_, val_ in pend:
                        v.wait_ge(sem_, val_)
                    pend = []
                last = None
                for kk in range(BLOCKS[b]):
                    t = starts[b] + kk
                    mslice = ob[:, slot + kk * FD: slot + (kk + 1) * FD]
                    xslice = xb[:, t * FD:(t + 1) * FD]
                    if t == 0:
                        # mem_0 = x_0 (u_{-1} = 0); gate on chunk-0 load
                        last = chain(v.tensor_copy(mslice, xslice))
                        last._wait_ge(xc[0], 16)
                    elif _FUSE:
                        last = chain(v._custom_dve(
                            lif_op, out=mslice, in0=prev_mem, in1=xslice,
                            s0=float(c2), s1=float(d),
                        ))
                        if pend:
                            sem_, val_ = pend.pop(0)
                            last._wait_ge(sem_, val_)
                    else:
                        ins1 = chain(v.scalar_tensor_tensor(
                            uu[:], prev_mem, float(c2), prev_mem,
                            mask_op, mybir.AluOpType.mult,
                        ))
                        if pend:
                            sem_, val_ = pend.pop(0)
                            ins1._wait_ge(sem_, val_)
                        last = chain(v.scalar_tensor_tensor(
                            mslice, uu[:], float(d), xslice,
                            mybir.AluOpType.mult, mybir.AluOpType.add,
                        ))
                        if pend:
                            sem_, val_ = pend.pop(0)
                            last._wait_ge(sem_, val_)
                    prev_mem = mslice
                # block-produced signal fires when the last op COMPLETES
                # (DVE datapath completes in issue order)
                if _USE_CHAIN:
                    v.sem_inc(mrd, 1)._wait_ge(dvc, nv)
                else:
                    last.then_inc(mrd, 1)
                    if _FUSE and b + 1 < NBL:
                        # chunk b+1 gate rides this block's last op
                        last._wait_ge(xc[b + 1], 16)

    # pack .instr bytes for InstISA subclasses (custom-DVE); raw Bass
    # doesn't run this pass and walrus rejects empty blobs
    mybir.codegen_inst_isa_subclasses(nc)
    return nc


_PROGRAM_CACHE: dict = {}


def kernel(inpt: np.ndarray, v_th: np.ndarray, v_decay: np.ndarray) -> np.ndarray:
    global LAST_RUN
    x = np.ascontiguousarray(np.asarray(inpt, dtype=np.float32))
    assert x.shape == (T, B, N), x.shape
    vth = np.float32(np.asarray(v_th))
    d = float(np.float32(np.asarray(v_decay)))
    b, spk_is_gt = spike_boundary(vth)

    key = (float(b), d, spk_is_gt, _F32OUT, _USE_CHAIN, _FUSE)
    if key not in _PROGRAM_CACHE:
        _PROGRAM_CACHE[key] = build_program(float(b), d, spk_is_gt)
    nc = _PROGRAM_CACHE[key]

    in_maps = []
    nbias = np.full((P, 1), -np.float32(b), dtype=np.float32)
    for k in range(NCORES):
        xk = x[:, k * BPC:(k + 1) * BPC, :].reshape(T, P, FD)
        xk = np.ascontiguousarray(xk.transpose(1, 0, 2)).reshape(P, T * FD)
        m = {"x": xk}
        if not _F32OUT:
            m["nbias"] = nbias
        in_maps.append(m)

    res = run_bass_kernel_spmd(
        nc, in_maps, core_ids=list(range(NCORES)), trace=_TRACE
    )
    LAST_RUN = res

    tail_steps = sum(BLOCKS[-NF32TAIL:])
    t0 = T - tail_steps
    spikes = np.empty((T, B, N), dtype=np.float32)
    for k in range(NCORES):
        if _F32OUT:
            mem = res.results[k]["m"].reshape(P, T, FD).transpose(1, 0, 2)
            mem = mem.reshape(T, BPC, N)
            cmp = (mem > b) if spk_is_gt else (mem < b)
        else:
            s = res.results[k]["m8"].astype(np.float32)
            s = s.reshape(P, T, FD).transpose(1, 0, 2).reshape(T, BPC, N)
            cmp = (s > 0) if spk_is_gt else (s < 0)
            mt = res.results[k]["mtail"].reshape(P, tail_steps, FD)
            mt = mt.transpose(1, 0, 2).reshape(tail_steps, BPC, N)
            cmp[t0:] = (mt > b) if spk_is_gt else (mt < b)
        spikes[:, k * BPC:(k + 1) * BPC, :] = cmp
    return spikes
